# revision 3
# baseline (speedup 1.0000x reference)
"""DeepSeek-V2-style MoE layer on 8 Trainium2 NeuronCores (Bass/Tile).

Strategy (expert-parallel, mirroring the reference's capacity dispatch):
  host:   gate (softmax + top-6, bit-matching the reference via jax-CPU),
          token->expert dispatch (sorted, capacity-dropped), weight
          pre-transposition/packing so every device GEMM is natural layout.
  device: each core owns 4 routed experts (snake-assigned by load) plus a
          (token-quarter x inter-half) shard of the shared MLP.  Per unit:
             out13[2816, C] = W13^T @ X          (fp32r matmuls)
             hT[1408, C]    = silu(out13[a]) * out13[b]   (fused on eviction)
             obT[2048, C]   = W2T^T @ hT
  host:   weighted scatter-add combine + shared partial reduction.

Matmuls run as float32r (full-rate fp32 path of the PE, ~1.5e-4 rel err)
with fp32 storage end-to-end.
"""

import os
import sys
from contextlib import ExitStack

import numpy as np

for _p in ("/opt/trn_rl_repo",):
    if os.path.isdir(_p) and _p not in sys.path:
        sys.path.insert(0, _p)

import concourse.bass as bass  # noqa: E402
import concourse.mybir as mybir  # noqa: E402
import concourse.tile as tile  # noqa: E402
from concourse import bacc  # noqa: E402
from concourse.bass_utils import run_bass_kernel_spmd  # noqa: E402

# ---- problem constants (hardcoded per contract) ----
B, NSEQ, D = 2, 2048, 2048
T = B * NSEQ                      # 4096 tokens
E, F = 32, 1408                   # experts, expert inter dim
S = 2 * F                         # 2816 shared inter dim
TOPK = 6
CAP = int(1.5 * T * TOPK / E)     # 1152: reference capacity (drops beyond)
NCORES = 8
RPC = E // NCORES                 # routed experts per core = 4
P = 128
KO = D // P                       # 16 k-strips of contraction over D
KF = F // P                       # 11 k-strips of contraction over F
MF = (2 * F) // 256               # 11 m-strips of W13 (256 cols = silu pair)
FREE = 512                        # PSUM free-dim tile
SH_TOK = T // 4                   # 1024 shared tokens per core (quarter)

MM_DT = mybir.dt.float32r         # matmul dtype (bitcast views of fp32)
F32 = mybir.dt.float32

# exec time of the last device launch (ns), when tracing was requested
last_exec_time_ns = None


def _gate(xf, wg):
    """Mirror the reference gate; jax-CPU for bit-identical top-k picks."""
    try:
        import jax
        import jax.numpy as jnp

        cpu = jax.devices("cpu")[0]
        with jax.default_device(cpu):
            logits = jnp.asarray(xf) @ jnp.asarray(wg).T
            probs = jax.nn.softmax(logits.astype(jnp.float32), axis=-1)
            _, idx = jax.lax.top_k(probs, TOPK)
            w = jnp.take_along_axis(probs, idx, axis=1)
            return np.asarray(idx), np.asarray(w)
    except Exception:
        logits = xf @ wg.T
        m = logits.max(axis=1, keepdims=True)
        p = np.exp(logits - m)
        probs = (p / p.sum(axis=1, keepdims=True)).astype(np.float32)
        idx = np.argsort(-probs, axis=1, kind="stable")[:, :TOPK]
        w = np.take_along_axis(probs, idx, axis=1)
        return idx.astype(np.int32), w


def _dispatch(idx, w):
    """Group token slots by expert in reference slot order, drop past CAP."""
    se = idx.reshape(-1)
    tid = np.repeat(np.arange(T), TOPK)
    wf = w.reshape(-1)
    order = np.argsort(se, kind="stable")
    se_s, tid_s, wf_s = se[order], tid[order], wf[order]
    counts = np.bincount(se, minlength=E)
    tok, wgt = [], []
    start = 0
    for e in range(E):
        n = int(counts[e])
        k = min(n, CAP)
        tok.append(tid_s[start:start + k])
        wgt.append(wf_s[start:start + k].astype(np.float32))
        start += n
    return tok, wgt


def _assign(tok):
    """Snake-assign experts to cores by descending load; uniform slot caps."""
    order = np.argsort([-len(t) for t in tok], kind="stable")
    slots = [[0] * RPC for _ in range(NCORES)]
    for r in range(RPC):
        grp = list(order[r * NCORES:(r + 1) * NCORES])
        if r % 2:
            grp = grp[::-1]
        for c in range(NCORES):
            slots[c][r] = int(grp[c])
    caps = []
    for r in range(RPC):
        m = max(len(tok[slots[c][r]]) for c in range(NCORES))
        caps.append(max(P, -(-m // P) * P))  # pad to 128, at least 128
    return slots, caps


def _nblocks(width):
    """Split width (mult of 128) into <=512 blocks, balanced to >=256."""
    nt = width // P
    k = -(-nt // 4)  # ceil(nt/4) blocks of <=512
    sizes = [P * (nt // k + (1 if i < nt % k else 0)) for i in range(k)]
    out, off = [], 0
    for s2 in sizes:
        out.append((off, s2))
        off += s2
    return out


def _pack13(a, b):
    """[F',D] pair -> [D, 2F'] with alternating 128-row blocks (silu pairs)."""
    fdim = a.shape[0]
    at = a.reshape(fdim // P, P, D).transpose(2, 0, 1)  # [D, F'/128, 128]
    bt = b.reshape(fdim // P, P, D).transpose(2, 0, 1)
    return np.stack([at, bt], axis=2).reshape(D, 2 * fdim)


def _build_program(caps):
    """Emit the SPMD per-core program. caps = routed slot widths (x4)."""
    widths = list(caps) + [SH_TOK]
    offs = np.concatenate([[0], np.cumsum(widths)]).astype(int)
    c_total = int(offs[-1])
    cmax = max(widths)

    nc = bacc.Bacc("TRN2", target_bir_lowering=False, debug=False,
                   num_devices=NCORES)
    x_all = nc.dram_tensor("x_all", [D, c_total], MM_DT, kind="ExternalInput").ap()
    w13 = nc.dram_tensor("w13", [RPC + 1, D, 2 * F], MM_DT, kind="ExternalInput").ap()
    w2t = nc.dram_tensor("w2t", [RPC + 1, F, D], MM_DT, kind="ExternalInput").ap()
    out = nc.dram_tensor("out", [D, c_total], F32, kind="ExternalOutput").ap()

    x_v = x_all.rearrange("(ko p) c -> p ko c", p=P)
    out_v = out.rearrange("(ko p) c -> p ko c", p=P)
    w13_v = w13.rearrange("e (ko p) m -> p ko e m", p=P)
    w2t_v = w2t.rearrange("e (kf p) d -> p kf e d", p=P)

    silu = mybir.ActivationFunctionType.Silu

    with tile.TileContext(nc) as tc:
        with ExitStack() as ctx:
            xp = ctx.enter_context(tc.tile_pool(name="xp", bufs=1))
            htp = ctx.enter_context(tc.tile_pool(name="htp", bufs=1))
            w13p = ctx.enter_context(tc.tile_pool(name="w13p", bufs=2))
            w2tp = ctx.enter_context(tc.tile_pool(name="w2tp", bufs=3))
            silp = ctx.enter_context(tc.tile_pool(name="silp", bufs=3))
            obp = ctx.enter_context(tc.tile_pool(name="obp", bufs=3))
            cstp = ctx.enter_context(tc.tile_pool(name="cstp", bufs=1))
            psa = ctx.enter_context(tc.tile_pool(name="psa", bufs=4, space="PSUM"))
            psb = ctx.enter_context(tc.tile_pool(name="psb", bufs=4, space="PSUM"))

            bias = cstp.tile([P, 1], F32)
            nc.any.memset(bias[:], 0.0)

            def load_x(u):
                off, width = int(offs[u]), int(widths[u])
                xt = xp.tile([P, KO, cmax], MM_DT, tag="X")
                for kc in range(0, KO, 4):
                    nc.sync.dma_start(xt[:, kc:kc + 4, :width],
                                      x_v[:, kc:kc + 4, off:off + width])
                return xt

            xt = load_x(0)
            for u in range(RPC + 1):
                widx = u if u < RPC else RPC
                off, width = int(offs[u]), int(widths[u])
                blocks = _nblocks(width)
                ht = htp.tile([P, KF, cmax], MM_DT, tag="ht")

                # ---- A: out13 = W13^T @ X, fused silu*mul -> ht ----
                for m in range(MF):
                    wt = w13p.tile([P, KO, 256], MM_DT, tag="w13")
                    nc.sync.dma_start(
                        wt[:], w13_v[:, :, widx, m * 256:(m + 1) * 256])
                    for nb_off, nb_w in blocks:
                        ps0 = psa.tile([P, FREE], F32, tag="psa")
                        ps1 = psa.tile([P, FREE], F32, tag="psa")
                        for k in range(KO):
                            nc.tensor.matmul(
                                ps0[:, :nb_w],
                                wt[:, k, 0:P],
                                xt[:, k, nb_off:nb_off + nb_w],
                                start=(k == 0), stop=(k == KO - 1))
                        for k in range(KO):
                            nc.tensor.matmul(
                                ps1[:, :nb_w],
                                wt[:, k, P:2 * P],
                                xt[:, k, nb_off:nb_off + nb_w],
                                start=(k == 0), stop=(k == KO - 1))
                        sil = silp.tile([P, FREE], F32, tag="sil")
                        nc.scalar.activation(sil[:, :nb_w], ps0[:, :nb_w],
                                             silu, bias=bias[:])
                        nc.vector.tensor_mul(
                            out=ht[:, m, nb_off:nb_off + nb_w],
                            in0=sil[:, :nb_w], in1=ps1[:, :nb_w])

                xt_next = load_x(u + 1) if u + 1 < RPC + 1 else None

                # ---- B: obT = W2T^T @ ht ----
                for dstrip in range(D // 256):
                    w2 = w2tp.tile([P, KF, 256], MM_DT, tag="w2t")
                    nc.sync.dma_start(
                        w2[:], w2t_v[:, :, widx, dstrip * 256:(dstrip + 1) * 256])
                    for din in range(2):
                        dcol = dstrip * 2 + din
                        for nb_off, nb_w in blocks:
                            ps = psb.tile([P, FREE], F32, tag="psb")
                            for k in range(KF):
                                nc.tensor.matmul(
                                    ps[:, :nb_w],
                                    w2[:, k, din * P:(din + 1) * P],
                                    ht[:, k, nb_off:nb_off + nb_w],
                                    start=(k == 0), stop=(k == KF - 1))
                            ob = obp.tile([P, FREE], F32, tag="ob")
                            nc.any.tensor_copy(out=ob[:, :nb_w], in_=ps[:, :nb_w])
                            nc.sync.dma_start(
                                out_v[:, dcol, off + nb_off:off + nb_off + nb_w],
                                ob[:, :nb_w])

                if xt_next is not None:
                    xt = xt_next

    nc.compile()
    return nc, offs, c_total


def kernel(x, wg, w1, w2, w3, ws1, ws2, ws3):
    x = np.ascontiguousarray(x, dtype=np.float32)
    xf = x.reshape(T, D)

    idx, gw = _gate(xf, np.asarray(wg, dtype=np.float32))
    tok, wgt = _dispatch(idx, gw)
    slots, caps = _assign(tok)

    nc, offs, c_total = _build_program(caps)

    w1 = np.asarray(w1, dtype=np.float32)
    w2 = np.asarray(w2, dtype=np.float32)
    w3 = np.asarray(w3, dtype=np.float32)
    ws1 = np.asarray(ws1, dtype=np.float32)
    ws2 = np.asarray(ws2, dtype=np.float32)
    ws3 = np.asarray(ws3, dtype=np.float32)

    in_maps = []
    for c in range(NCORES):
        half = c % 2
        quarter = c // 2
        x_core = np.zeros((D, c_total), np.float32)
        for r in range(RPC):
            tk = tok[slots[c][r]]
            x_core[:, offs[r]:offs[r] + len(tk)] = xf[tk].T
        x_core[:, offs[RPC]:offs[RPC] + SH_TOK] = \
            xf[quarter * SH_TOK:(quarter + 1) * SH_TOK].T

        w13_core = np.empty((RPC + 1, D, 2 * F), np.float32)
        w2t_core = np.empty((RPC + 1, F, D), np.float32)
        for r in range(RPC):
            e = slots[c][r]
            w13_core[r] = _pack13(w1[e], w3[e])
            w2t_core[r] = w2[e].T
        lo, hi = half * F, (half + 1) * F
        w13_core[RPC] = _pack13(ws1[lo:hi], ws3[lo:hi])
        w2t_core[RPC] = ws2[:, lo:hi].T
        in_maps.append({"x_all": x_core, "w13": w13_core, "w2t": w2t_core})

    trace = bool(int(os.environ.get("MOE_TRACE", "0")))
    res = run_bass_kernel_spmd(nc, in_maps, list(range(NCORES)), trace=trace)
    global last_exec_time_ns
    last_exec_time_ns = res.exec_time_ns

    y = np.zeros((T, D), np.float32)
    for c in range(NCORES):
        o = res.results[c]["out"]
        for r in range(RPC):
            e = slots[c][r]
            tk, wk = tok[e], wgt[e]
            if len(tk):
                y[tk] += o[:, offs[r]:offs[r] + len(tk)].T * wk[:, None]
    for q in range(4):
        sh = int(offs[RPC])
        part = res.results[2 * q]["out"][:, sh:sh + SH_TOK] \
            + res.results[2 * q + 1]["out"][:, sh:sh + SH_TOK]
        y[q * SH_TOK:(q + 1) * SH_TOK] += part.T

    return y.reshape(B, NSEQ, D), np.zeros((), np.float32)


# revision 4
# speedup vs baseline: 1.1313x; 1.1313x over previous
"""DeepSeek-V2-style MoE layer on 8 Trainium2 NeuronCores (Bass/Tile).

Strategy (expert-parallel, mirroring the reference's capacity dispatch):
  host:   gate (softmax + top-6, bit-matching the reference via jax-CPU),
          token->expert dispatch (sorted, capacity-dropped), weight
          pre-transposition/packing so every device GEMM is natural layout.
  device: each core owns 4 routed experts (snake-assigned by load) plus a
          (token-quarter x inter-half) shard of the shared MLP.  Per unit:
             out13[2816, C] = W13^T @ X          (fp32r matmuls)
             hT[1408, C]    = silu(out13[a]) * out13[b]   (fused on eviction)
             obT[2048, C]   = W2T^T @ hT
  host:   weighted scatter-add combine + shared partial reduction.

Matmuls run as float32r (full-rate fp32 path of the PE, ~1.5e-4 rel err)
with fp32 storage end-to-end.
"""

import os
import sys
from contextlib import ExitStack

import numpy as np

for _p in ("/opt/trn_rl_repo",):
    if os.path.isdir(_p) and _p not in sys.path:
        sys.path.insert(0, _p)

import concourse.bass as bass  # noqa: E402
import concourse.mybir as mybir  # noqa: E402
import concourse.tile as tile  # noqa: E402
from concourse import bacc  # noqa: E402
from concourse.bass_utils import run_bass_kernel_spmd  # noqa: E402

# ---- problem constants (hardcoded per contract) ----
B, NSEQ, D = 2, 2048, 2048
T = B * NSEQ                      # 4096 tokens
E, F = 32, 1408                   # experts, expert inter dim
S = 2 * F                         # 2816 shared inter dim
TOPK = 6
CAP = int(1.5 * T * TOPK / E)     # 1152: reference capacity (drops beyond)
NCORES = 8
RPC = E // NCORES                 # routed experts per core = 4
P = 128
KO = D // P                       # 16 k-strips of contraction over D
KF = F // P                       # 11 k-strips of contraction over F
MF = (2 * F) // 256               # 11 m-strips of W13 (256 cols = silu pair)
FREE = 512                        # PSUM free-dim tile
SH_TOK = T // 4                   # 1024 shared tokens per core (quarter)

MM_DT = mybir.dt.float32r         # matmul dtype (bitcast views of fp32)
F32 = mybir.dt.float32

# exec time of the last device launch (ns), when tracing was requested
last_exec_time_ns = None


def _gate(xf, wg):
    """Mirror the reference gate; jax-CPU for bit-identical top-k picks."""
    try:
        import jax
        import jax.numpy as jnp

        cpu = jax.devices("cpu")[0]
        with jax.default_device(cpu):
            logits = jnp.asarray(xf) @ jnp.asarray(wg).T
            probs = jax.nn.softmax(logits.astype(jnp.float32), axis=-1)
            _, idx = jax.lax.top_k(probs, TOPK)
            w = jnp.take_along_axis(probs, idx, axis=1)
            return np.asarray(idx), np.asarray(w)
    except Exception:
        logits = xf @ wg.T
        m = logits.max(axis=1, keepdims=True)
        p = np.exp(logits - m)
        probs = (p / p.sum(axis=1, keepdims=True)).astype(np.float32)
        idx = np.argsort(-probs, axis=1, kind="stable")[:, :TOPK]
        w = np.take_along_axis(probs, idx, axis=1)
        return idx.astype(np.int32), w


def _dispatch(idx, w):
    """Group token slots by expert in reference slot order, drop past CAP."""
    se = idx.reshape(-1)
    tid = np.repeat(np.arange(T), TOPK)
    wf = w.reshape(-1)
    order = np.argsort(se, kind="stable")
    se_s, tid_s, wf_s = se[order], tid[order], wf[order]
    counts = np.bincount(se, minlength=E)
    tok, wgt = [], []
    start = 0
    for e in range(E):
        n = int(counts[e])
        k = min(n, CAP)
        tok.append(tid_s[start:start + k])
        wgt.append(wf_s[start:start + k].astype(np.float32))
        start += n
    return tok, wgt


def _assign(tok):
    """Snake-assign experts to cores by descending load; uniform slot caps."""
    order = np.argsort([-len(t) for t in tok], kind="stable")
    slots = [[0] * RPC for _ in range(NCORES)]
    for r in range(RPC):
        grp = list(order[r * NCORES:(r + 1) * NCORES])
        if r % 2:
            grp = grp[::-1]
        for c in range(NCORES):
            slots[c][r] = int(grp[c])
    caps = []
    for r in range(RPC):
        m = max(len(tok[slots[c][r]]) for c in range(NCORES))
        caps.append(max(P, -(-m // P) * P))  # pad to 128, at least 128
    return slots, caps


def _nblocks(width):
    """Split width (mult of 128) into <=512 blocks, balanced to >=256."""
    nt = width // P
    k = -(-nt // 4)  # ceil(nt/4) blocks of <=512
    sizes = [P * (nt // k + (1 if i < nt % k else 0)) for i in range(k)]
    out, off = [], 0
    for s2 in sizes:
        out.append((off, s2))
        off += s2
    return out


def _pack13(a, b):
    """[F',D] pair -> [D, 2F'] with alternating 128-row blocks (silu pairs)."""
    fdim = a.shape[0]
    at = a.reshape(fdim // P, P, D).transpose(2, 0, 1)  # [D, F'/128, 128]
    bt = b.reshape(fdim // P, P, D).transpose(2, 0, 1)
    return np.stack([at, bt], axis=2).reshape(D, 2 * fdim)


def _build_program(caps):
    """Emit the SPMD per-core program. caps = routed slot widths (x4)."""
    widths = list(caps) + [SH_TOK]
    offs = np.concatenate([[0], np.cumsum(widths)]).astype(int)
    c_total = int(offs[-1])
    cmax = max(widths)

    nc = bacc.Bacc("TRN2", target_bir_lowering=False, debug=False,
                   num_devices=NCORES)
    x_all = nc.dram_tensor("x_all", [D, c_total], MM_DT, kind="ExternalInput").ap()
    w13 = nc.dram_tensor("w13", [RPC + 1, D, 2 * F], MM_DT, kind="ExternalInput").ap()
    w2t = nc.dram_tensor("w2t", [RPC + 1, F, D], MM_DT, kind="ExternalInput").ap()
    out = nc.dram_tensor("out", [D, c_total], F32, kind="ExternalOutput").ap()

    x_v = x_all.rearrange("(ko p) c -> p ko c", p=P)
    out_v = out.rearrange("(ko p) c -> p ko c", p=P)
    w13_v = w13.rearrange("e (ko p) m -> p ko e m", p=P)
    w2t_v = w2t.rearrange("e (kf p) d -> p kf e d", p=P)

    silu = mybir.ActivationFunctionType.Silu

    with tile.TileContext(nc) as tc:
        with ExitStack() as ctx:
            xp = ctx.enter_context(tc.tile_pool(name="xp", bufs=1))
            htp = ctx.enter_context(tc.tile_pool(name="htp", bufs=1))
            w13p = ctx.enter_context(tc.tile_pool(name="w13p", bufs=2))
            w2tp = ctx.enter_context(tc.tile_pool(name="w2tp", bufs=2))
            silp = ctx.enter_context(tc.tile_pool(name="silp", bufs=3))
            obp = ctx.enter_context(tc.tile_pool(name="obp", bufs=3))
            cstp = ctx.enter_context(tc.tile_pool(name="cstp", bufs=1))
            psa = ctx.enter_context(tc.tile_pool(name="psa", bufs=4, space="PSUM"))
            psb = ctx.enter_context(tc.tile_pool(name="psb", bufs=4, space="PSUM"))

            bias = cstp.tile([P, 1], F32)
            nc.any.memset(bias[:], 0.0)

            for u in range(RPC + 1):
                widx = u if u < RPC else RPC
                off, width = int(offs[u]), int(widths[u])
                blocks = _nblocks(width)

                xt = xp.tile([P, KO, cmax], MM_DT, tag="X")
                nc.sync.dma_start(xt[:, :, :width], x_v[:, :, off:off + width])
                ht = htp.tile([P, KF, cmax], MM_DT, tag="ht")

                # ---- A: out13 = W13^T @ X, fused silu*mul -> ht ----
                for m in range(MF):
                    wt = w13p.tile([P, KO, 256], MM_DT, tag="w13")
                    nc.sync.dma_start(
                        wt[:], w13_v[:, :, widx, m * 256:(m + 1) * 256])
                    for nb_off, nb_w in blocks:
                        ps0 = psa.tile([P, FREE], F32, tag="psa")
                        ps1 = psa.tile([P, FREE], F32, tag="psa")
                        for k in range(KO):
                            nc.tensor.matmul(
                                ps0[:, :nb_w],
                                wt[:, k, 0:P],
                                xt[:, k, nb_off:nb_off + nb_w],
                                start=(k == 0), stop=(k == KO - 1))
                        for k in range(KO):
                            nc.tensor.matmul(
                                ps1[:, :nb_w],
                                wt[:, k, P:2 * P],
                                xt[:, k, nb_off:nb_off + nb_w],
                                start=(k == 0), stop=(k == KO - 1))
                        sil = silp.tile([P, FREE], F32, tag="sil")
                        nc.scalar.activation(sil[:, :nb_w], ps0[:, :nb_w],
                                             silu, bias=bias[:])
                        nc.vector.tensor_mul(
                            out=ht[:, m, nb_off:nb_off + nb_w],
                            in0=sil[:, :nb_w], in1=ps1[:, :nb_w])

                # ---- B: obT = W2T^T @ ht ----
                for dstrip in range(D // 256):
                    w2 = w2tp.tile([P, KF, 256], MM_DT, tag="w2t")
                    nc.sync.dma_start(
                        w2[:], w2t_v[:, :, widx, dstrip * 256:(dstrip + 1) * 256])
                    for din in range(2):
                        dcol = dstrip * 2 + din
                        for nb_off, nb_w in blocks:
                            ps = psb.tile([P, FREE], F32, tag="psb")
                            for k in range(KF):
                                nc.tensor.matmul(
                                    ps[:, :nb_w],
                                    w2[:, k, din * P:(din + 1) * P],
                                    ht[:, k, nb_off:nb_off + nb_w],
                                    start=(k == 0), stop=(k == KF - 1))
                            ob = obp.tile([P, FREE], F32, tag="ob")
                            nc.any.tensor_copy(out=ob[:, :nb_w], in_=ps[:, :nb_w])
                            nc.sync.dma_start(
                                out_v[:, dcol, off + nb_off:off + nb_off + nb_w],
                                ob[:, :nb_w])

    nc.compile()
    return nc, offs, c_total


def kernel(x, wg, w1, w2, w3, ws1, ws2, ws3):
    x = np.ascontiguousarray(x, dtype=np.float32)
    xf = x.reshape(T, D)

    idx, gw = _gate(xf, np.asarray(wg, dtype=np.float32))
    tok, wgt = _dispatch(idx, gw)
    slots, caps = _assign(tok)

    nc, offs, c_total = _build_program(caps)

    w1 = np.asarray(w1, dtype=np.float32)
    w2 = np.asarray(w2, dtype=np.float32)
    w3 = np.asarray(w3, dtype=np.float32)
    ws1 = np.asarray(ws1, dtype=np.float32)
    ws2 = np.asarray(ws2, dtype=np.float32)
    ws3 = np.asarray(ws3, dtype=np.float32)

    in_maps = []
    for c in range(NCORES):
        half = c % 2
        quarter = c // 2
        x_core = np.zeros((D, c_total), np.float32)
        for r in range(RPC):
            tk = tok[slots[c][r]]
            x_core[:, offs[r]:offs[r] + len(tk)] = xf[tk].T
        x_core[:, offs[RPC]:offs[RPC] + SH_TOK] = \
            xf[quarter * SH_TOK:(quarter + 1) * SH_TOK].T

        w13_core = np.empty((RPC + 1, D, 2 * F), np.float32)
        w2t_core = np.empty((RPC + 1, F, D), np.float32)
        for r in range(RPC):
            e = slots[c][r]
            w13_core[r] = _pack13(w1[e], w3[e])
            w2t_core[r] = w2[e].T
        lo, hi = half * F, (half + 1) * F
        w13_core[RPC] = _pack13(ws1[lo:hi], ws3[lo:hi])
        w2t_core[RPC] = ws2[:, lo:hi].T
        in_maps.append({"x_all": x_core, "w13": w13_core, "w2t": w2t_core})

    trace = bool(int(os.environ.get("MOE_TRACE", "0")))
    res = run_bass_kernel_spmd(nc, in_maps, list(range(NCORES)), trace=trace)
    global last_exec_time_ns
    last_exec_time_ns = res.exec_time_ns

    y = np.zeros((T, D), np.float32)
    for c in range(NCORES):
        o = res.results[c]["out"]
        for r in range(RPC):
            e = slots[c][r]
            tk, wk = tok[e], wgt[e]
            if len(tk):
                y[tk] += o[:, offs[r]:offs[r] + len(tk)].T * wk[:, None]
    for q in range(4):
        sh = int(offs[RPC])
        part = res.results[2 * q]["out"][:, sh:sh + SH_TOK] \
            + res.results[2 * q + 1]["out"][:, sh:sh + SH_TOK]
        y[q * SH_TOK:(q + 1) * SH_TOK] += part.T

    return y.reshape(B, NSEQ, D), np.zeros((), np.float32)
